# revision 1
# baseline (speedup 1.0000x reference)
"""Causal self-attention (B=2, T=2048, C=1024, NH=16, HD=64) on 8 NeuronCores.

Sharding: core c -> (batch b = c//4, head-group hg = c%4 of 4 heads).
Each core computes qkv projection for its 4 heads from x[b], attention for
its 4 (b,h) units, and a partial output projection (row-parallel over the
head dim). Unshard = sum of the 4 partials per batch (bproj/4 folded in).

Per-core device algorithm (all fp32):
  A. x[b] transposed on PE (128x128 tiles) -> xT [1024, 2048] in SBUF.
  B. qkT [512, 2048] = wqk.T @ xT (+bias at evict, per-partition add).
     Rows 0..255 = qT of 4 heads, 256..511 = kT.
  C. v_aug [2048, 260] = [x[b] | 1] @ wv_aug: per head 64 v columns + a
     ones column (from the aug bias row) used to compute softmax sums as a
     matmul byproduct; v bias also folded into the aug row.
  D. Per (head s, i-chunk ic of 512): att^T[j,i] blocks via PE (K=64),
     exp on ACT (scale = 1/8, no max subtraction -- logits are O(1) by
     construction), causal mask = multiplicative sliding mask on diagonal
     blocks, y^T accumulation [65, 512] where row 64 = softmax sums S.
     Normalize: recip(S) -> PE ones-broadcast to [64,512] -> DVE mul.
  E. out_partial [2048, 1024] = yT.T @ wp + bproj/4, DMA to DRAM.
"""
import os
import sys

import numpy as np

for _p in ("/opt/trn_rl_repo",):
    if _p not in sys.path and os.path.isdir(_p):
        sys.path.insert(0, _p)

import concourse.bass as bass
import concourse.mybir as mybir
import concourse.tile as tile
from concourse.masks import make_identity

B, T, C, NH, HD = 2, 2048, 1024, 16, 64
F32 = mybir.dt.float32
F32R = mybir.dt.float32r
N_CORES = 8
NT = T // 128   # 16 token tiles
NQ = 4          # token quarters (512 tokens each)
NKT = C // 128  # 8 contraction tiles


def build_nc(split_waits=True):
    nc = bass.Bass()
    xb_d = nc.declare_dram_parameter("xb", [T, C], F32R, isOutput=False)
    wqk_d = nc.declare_dram_parameter("wqk", [C, 512], F32R, isOutput=False)
    bqk_d = nc.declare_dram_parameter("bqk", [512], F32, isOutput=False)
    wv_d = nc.declare_dram_parameter("wv", [C + 1, 260], F32R, isOutput=False)
    wp_d = nc.declare_dram_parameter("wp", [256, C], F32R, isOutput=False)
    bp4_d = nc.declare_dram_parameter("bp4", [128, C], F32, isOutput=False)
    out_d = nc.declare_dram_parameter("out", [T, C], F32, isOutput=True)

    with tile.TileContext(nc) as tc:
        with (
            tc.tile_pool(name="const", bufs=1) as const,
            tc.tile_pool(name="wts", bufs=1) as wts,
            tc.tile_pool(name="xbp", bufs=4) as xbp,
            tc.tile_pool(name="xtp", bufs=2) as xtp,
            tc.tile_pool(name="qkt", bufs=1) as qkt,
            tc.tile_pool(name="vsb", bufs=1) as vsb,
            tc.tile_pool(name="ep", bufs=6) as ep,
            tc.tile_pool(name="rsp", bufs=3) as rsp,
            tc.tile_pool(name="ytp", bufs=1) as ytp,
            tc.tile_pool(name="outp", bufs=3) as outp,
            tc.tile_pool(name="psA", bufs=2, space="PSUM") as psA,
            tc.tile_pool(name="psE", bufs=2, space="PSUM") as psE,
            tc.tile_pool(name="psY", bufs=2, space="PSUM") as psY,
        ):
            psR = psA
            # ---- constants ----
            # gpsimd can't touch f32r; build f32 versions then DVE copy-cast.
            ident32 = const.tile([128, 128], F32, name="ident32")
            make_identity(nc, ident32)
            ident = const.tile([128, 128], F32R, name="ident")
            nc.vector.tensor_copy(ident[:], ident32[:])
            # sliding causal multiplicative mask: maskf[j, c] = 1 iff c >= j + 384
            maskf32 = const.tile([128, 896], F32, name="maskf32")
            nc.gpsimd.memset(maskf32[:], 1.0)
            nc.gpsimd.affine_select(
                out=maskf32[:], in_=maskf32[:],
                compare_op=mybir.AluOpType.is_ge, fill=0.0,
                base=-384, channel_multiplier=-1, pattern=[[1, 896]],
            )
            maskf = const.tile([128, 896], F32R, name="maskf")
            nc.vector.tensor_copy(maskf[:], maskf32[:])
            ones32 = const.tile([1, 128], F32, name="ones32")
            nc.gpsimd.memset(ones32[:], 1.0)
            ones_r = const.tile([1, 128], F32R, name="ones_r")
            nc.vector.tensor_copy(ones_r[:], ones32[:])
            bqk_sb = const.tile([128, 4], F32, name="bqk_sb")
            nc.sync.dma_start(out=bqk_sb[:], in_=bqk_d.rearrange("(t p) -> p t", p=128))

            # ---- first-quarter x loads issued before weights (critical path) ----
            xb_q0 = []
            for q in range(4):
                t = xbp.tile([128, C], F32R, name="xb_t", tag="xb_t")
                nc.sync.dma_start(out=t[:], in_=xb_d[q * 128:(q + 1) * 128, :])
                xb_q0.append(t)

            # ---- weights ----
            wqk_sb = []
            for kt in range(NKT):
                w = wts.tile([128, 512], F32R, name=f"wqk{kt}", tag=f"wqk{kt}")
                nc.sync.dma_start(out=w[:], in_=wqk_d[kt * 128:(kt + 1) * 128, :])
                wqk_sb.append(w)
            wv_sb = []
            for kt in range(NKT):
                w = wts.tile([128, 260], F32R, name=f"wv{kt}", tag=f"wv{kt}")
                nc.sync.dma_start(out=w[:], in_=wv_d[kt * 128:(kt + 1) * 128, :])
                wv_sb.append(w)
            wv_last = wts.tile([1, 260], F32R, name="wv_last", tag="wv_last")
            nc.sync.dma_start(out=wv_last[:], in_=wv_d[C:C + 1, :])
            wp_sb = []
            for kt in range(2):
                w = wts.tile([128, C], F32R, name=f"wp{kt}", tag=f"wp{kt}")
                nc.sync.dma_start(out=w[:], in_=wp_d[kt * 128:(kt + 1) * 128, :])
                wp_sb.append(w)
            bp4_sb = const.tile([128, C], F32, name="bp4_sb")
            nc.sync.dma_start(out=bp4_sb[:], in_=bp4_d[:])

            # ---- persistent activations ----
            qkT = [qkt.tile([128, T], F32R, name=f"qkT{p}", tag=f"qkT{p}")
                   for p in range(4)]
            v_sb = [vsb.tile([128, 260], F32R, name=f"v{jt}", tag=f"v{jt}")
                    for jt in range(NT)]
            yT = [[ytp.tile([128, 512], F32R, name=f"yT{ic}_{kt}",
                            tag=f"yT{ic}_{kt}") for kt in range(2)]
                  for ic in range(NQ)]

            # ---- stages D+E for one quarter (called inside the Q loop
            # so exp/mask work interleaves with the PE-heavy A/B/C) ----
            def attn_quarter(ic):
                for s in range(4):
                    qrow = (s % 2) * 64
                    qtile = qkT[s // 2]
                    ktile = qkT[2 + s // 2]
                    n_jt = 4 * ic + 4
                    ps_y = psY.tile([65, 512], F32, name="ps_y", tag="psY")

                    # y accumulation order: diagonal band first (jt=4ic has
                    # o=0, full width, carries start), then the full band;
                    # the last emitted matmul carries stop.
                    n_y = 0

                    def att_block(jt, h, ps_a, e, o):
                        # o = column offset below which everything is masked
                        nc.tensor.matmul(
                            ps_a[:, h * 512 + o:(h + 1) * 512],
                            ktile[qrow:qrow + 64, jt * 128:(jt + 1) * 128],
                            qtile[qrow:qrow + 64,
                                  ic * 512 + o:(ic + 1) * 512],
                            start=True, stop=True,
                        )
                        return (jt, h, o, e)

                    def finish_block(jt, h, o, e):
                        nonlocal n_y
                        if jt >= 4 * ic:
                            nc.vector.tensor_mul(
                                e[:, h * 512 + o:(h + 1) * 512],
                                e[:, h * 512 + o:(h + 1) * 512],
                                maskf[:, 384: 896 - o])
                        nc.tensor.matmul(
                            ps_y[:, o:512], v_sb[jt][:, s * 65:s * 65 + 65],
                            e[:, h * 512 + o:(h + 1) * 512],
                            start=(n_y == 0), stop=(n_y == n_jt - 1),
                        )
                        n_y += 1

                    # diagonal band: 4 partially-masked tiles; compute/exp
                    # only the live columns [o:512)
                    for half in range(2):
                        ps_a = psE.tile([128, 1024], F32, name="ps_a", tag="psE")
                        e = ep.tile([128, 1024], F32R, name="e_t", tag="e_t")
                        blocks = []
                        for h in range(2):
                            jt = 4 * ic + 2 * half + h
                            o = (2 * half + h) * 128
                            blocks.append(att_block(jt, h, ps_a, e, o))
                            nc.scalar.activation(
                                e[:, h * 512 + o:(h + 1) * 512],
                                ps_a[:, h * 512 + o:(h + 1) * 512],
                                mybir.ActivationFunctionType.Exp,
                                scale=float(HD) ** -0.5)
                        for blk in blocks:
                            finish_block(*blk)
                    # full band: pairs of j-tiles share a 2-bank psum so exp
                    # runs once per pair (halves ACT instruction count)
                    for pr in range(2 * ic):
                        ps_a = psE.tile([128, 1024], F32, name="ps_a", tag="psE")
                        e = ep.tile([128, 1024], F32R, name="e_t", tag="e_t")
                        blocks = [att_block(2 * pr + h, h, ps_a, e, 0)
                                  for h in range(2)]
                        nc.scalar.activation(
                            e[:], ps_a[:], mybir.ActivationFunctionType.Exp,
                            scale=float(HD) ** -0.5)
                        for blk in blocks:
                            finish_block(*blk)

                    rs = rsp.tile([1, 512], F32R, name="rs_t", tag="rs_t")
                    with nc.allow_low_precision(
                            reason="f32r is bit-identical to f32 on DVE"):
                        nc.vector.reciprocal(rs[:], ps_y[64:65, :])
                    ps_r = psR.tile([64, 512], F32, name="ps_r", tag="psA")
                    nc.tensor.matmul(ps_r[:], ones_r[:, :64], rs[:],
                                     start=True, stop=True)
                    r_sb = rsp.tile([64, 512], F32, name="r_sb", tag="r_sb")
                    nc.scalar.copy(r_sb[:], ps_r[:])
                    nc.vector.tensor_mul(
                        yT[ic][s // 2][qrow:qrow + 64, :],
                        ps_y[0:64, :], r_sb[:])

                # ---- stage E for this quarter: projection partial ----
                for mtl in range(4):
                    mt = 4 * ic + mtl
                    o = outp.tile([128, C], F32, name="o_t", tag="o_t")
                    for nch in range(2):
                        ps = psA.tile([128, 512], F32, name="ps_o", tag="psA")
                        for kt in range(2):
                            nc.tensor.matmul(
                                ps[:], yT[ic][kt][:, mtl * 128:(mtl + 1) * 128],
                                wp_sb[kt][:, nch * 512:(nch + 1) * 512],
                                start=(kt == 0), stop=(kt == 1),
                            )
                        nc.vector.tensor_add(
                            o[:, nch * 512:(nch + 1) * 512], ps[:],
                            bp4_sb[:, nch * 512:(nch + 1) * 512])
                        nc.sync.dma_start(
                            out=out_d[mt * 128:(mt + 1) * 128,
                                      nch * 512:(nch + 1) * 512],
                            in_=o[:, nch * 512:(nch + 1) * 512])


            # ---- stages A/B/C per token-quarter ----
            for Q in range(NQ):
                if Q == 0:
                    xb_sb = xb_q0
                else:
                    xb_sb = []
                    for q in range(4):
                        jt = 4 * Q + q
                        t = xbp.tile([128, C], F32R, name="xb_t", tag="xb_t")
                        nc.sync.dma_start(out=t[:],
                                          in_=xb_d[jt * 128:(jt + 1) * 128, :])
                        xb_sb.append(t)
                # A: transpose quarter -> xT_q[kt] [128, 512]
                xT_q = []
                for kt in range(NKT):
                    xt = xtp.tile([128, 512], F32R, name="xT_t", tag=f"xT{kt}")
                    ps = psA.tile([128, 512], F32R, name="ps_tr", tag="psA")
                    for q in range(4):
                        nc.tensor.transpose(
                            ps[:, q * 128:(q + 1) * 128],
                            xb_sb[q][:, kt * 128:(kt + 1) * 128],
                            ident[:],
                        )
                    if kt % 2 == 0:
                        nc.scalar.copy(xt[:], ps[:])
                    else:
                        nc.vector.tensor_copy(xt[:], ps[:])
                    xT_q.append(xt)
                # B: qkT chunk [512 part, 512 tokens]
                for p in range(4):
                    ps = psA.tile([128, 512], F32, name="ps_qk", tag="psA")
                    for kt in range(NKT):
                        nc.tensor.matmul(
                            ps[:], wqk_sb[kt][:, p * 128:(p + 1) * 128], xT_q[kt][:],
                            start=(kt == 0), stop=(kt == NKT - 1),
                        )
                    nc.vector.tensor_scalar_add(
                        qkT[p][:, Q * 512:(Q + 1) * 512], ps[:], bqk_sb[:, p:p + 1])
                # C: v_aug tiles [128, 260]
                for mt in range(4):
                    jt = 4 * Q + mt
                    ps = psA.tile([128, 260], F32, name="ps_v", tag="psA")
                    for kt in range(NKT):
                        nc.tensor.matmul(
                            ps[:], xT_q[kt][:, mt * 128:(mt + 1) * 128], wv_sb[kt][:],
                            start=(kt == 0), stop=False,
                        )
                    nc.tensor.matmul(ps[:], ones_r[:], wv_last[:],
                                     start=False, stop=True)
                    nc.vector.tensor_copy(v_sb[jt][:], ps[:])

                if Q >= 1:
                    attn_quarter(Q - 1)


            attn_quarter(NQ - 1)

    if split_waits:
        _split_matmul_waits(nc)
    return nc


def _split_matmul_waits(nc):
    """Walrus codegen in this pipeline allows only one sync wait per
    instruction for most ISA structs (S3_LW, PSEUDO_DMA_DIRECT2D, S3D3_TS,
    ...). Move extra waits onto inserted NoOps on the same engine (program
    order preserves semantics)."""
    n_split = 0
    for bb in nc.main_func.blocks:
        out = []
        for ins in bb.instructions:
            si = getattr(ins, "sync_info", None)
            if (si is not None and len(si.on_wait) >= 2
                    and type(ins).__name__ != "InstNoOp"):
                for w in si.on_wait[:-1]:
                    nop = mybir.InstNoOp(name=f"I-wsplit-{nc.next_id()}",
                                         ins=[], outs=[])
                    nop.engine = ins.engine
                    nop.sync_info = mybir.SyncInfo(on_wait=[w], on_update=[])
                    out.append(nop)
                    n_split += 1
                ins.sync_info = mybir.SyncInfo(
                    on_wait=[si.on_wait[-1]], on_update=si.on_update)
            out.append(ins)
        bb.instructions[:] = out
    return n_split


def shard_inputs(x, Wqkv, bqkv, Wproj, bproj):
    x = np.ascontiguousarray(np.asarray(x, np.float32))
    Wqkv = np.asarray(Wqkv, np.float32)
    bqkv = np.asarray(bqkv, np.float32)
    Wproj = np.asarray(Wproj, np.float32)
    bproj = np.asarray(bproj, np.float32)
    in_maps = []
    for c in range(N_CORES):
        b, hg = c // 4, c % 4
        wqk = np.ascontiguousarray(np.concatenate(
            [Wqkv[:, hg * 256:(hg + 1) * 256],
             Wqkv[:, C + hg * 256: C + (hg + 1) * 256]], axis=1))
        bqk = np.ascontiguousarray(np.concatenate(
            [bqkv[hg * 256:(hg + 1) * 256],
             bqkv[C + hg * 256: C + (hg + 1) * 256]]))
        wv = np.zeros((C + 1, 260), np.float32)
        for s in range(4):
            h = 4 * hg + s
            wv[:C, s * 65:s * 65 + 64] = Wqkv[:, 2 * C + h * 64: 2 * C + (h + 1) * 64]
            wv[C, s * 65:s * 65 + 64] = bqkv[2 * C + h * 64: 2 * C + (h + 1) * 64]
            wv[C, s * 65 + 64] = 1.0
        wp = np.ascontiguousarray(Wproj[hg * 256:(hg + 1) * 256, :])
        bp4 = np.ascontiguousarray(np.tile((bproj / 4)[None, :], (128, 1)))
        in_maps.append({"xb": x[b], "wqk": wqk, "bqk": bqk, "wv": wv,
                        "wp": wp, "bp4": bp4})
    return in_maps


_NC_CACHE = {}


def kernel(x, Wqkv, bqkv, Wproj, bproj):
    from concourse.bass_utils import run_bass_kernel_spmd

    if "nc" not in _NC_CACHE:
        _NC_CACHE["nc"] = build_nc()
    nc = _NC_CACHE["nc"]
    in_maps = shard_inputs(x, Wqkv, bqkv, Wproj, bproj)
    res = run_bass_kernel_spmd(nc, in_maps, list(range(N_CORES)))
    _NC_CACHE["last_exec_time_ns"] = res.exec_time_ns
    out = np.zeros((B, T, C), np.float32)
    for c in range(N_CORES):
        out[c // 4] += res.results[c]["out"]
    return out

